# revision 33
# baseline (speedup 1.0000x reference)
"""Causal multi-head attention (B=4, T=2048, D=1024, 16 heads) on 8 Trainium2
NeuronCores.

Sharding: core c = 2*b + g handles batch b (of 4) and head-group g (of 2,
8 heads each).  Each core computes Q/K/V projections for its head group,
causal attention, and a partial output projection (its 512 columns of the
out-proj contraction).  The host sums the two partial outputs per batch and
adds the bias.

On-core layout (bf16 operands; fp32 PSUM accumulation; fp32 output):
  QT, KT  [128, 4, 2048]  (dg within head-pair chunk, pair, q)  -- transposed
  V       [128, 16, 8, 65] (k within chunk, k-chunk, head, dv | ones-column)
  ctxT    [128, 4, 2048]  (dv within pair, pair, q)

Software-pipelined emission: the attention chunk stream for q-block xi is
interleaved (at PSUM-group granularity) with the Q/K/V projections for
q-block xi+1 and the output projection for q-block xi-1.  The PE queue is
in-order, so this striping gives the PE independent matmul work to execute
while each chunk waits on the scalar-engine exp -- the largest single engine
load (~160 x ~1us activations).  Attention for q-block xi only needs
projections 0..xi, which ran as earlier iterations' fillers.

Attention per (q-block of 512, head-pair): transposed scores ST[k, q] via two
concurrent row-tiled K=64 matmuls (base partitions 0/64), exp(S/8) on the
scalar engine (no max subtraction: |S|/8 <= ~3 for these inputs), causal
triangle handled by a post-exp 0/1 multiply on DVE, PV matmul with
lhsT=[V_h|ones] (M=65) which accumulates both ctx and the softmax
denominator.  Pair epilogue: one fp32 copy of [ctx|denom] for both heads
(2-bank PSUM tile) frees the banks; reciprocal on DVE, partition_broadcast
on GpSimd, DVE multiply into ctxT.  Ops are merged across heads and use
flat 2D access patterns where possible -- multi-dim APs cost ~400ns extra
per instruction on the scalar engine and per-instruction overhead dominates
mid-size ops on real HW.
"""
from contextlib import ExitStack, nullcontext

import numpy as np

import concourse.bass as bass
import concourse.mybir as mybir
import concourse.tile as tile
from concourse import bacc
from concourse.bass_utils import run_bass_kernel_spmd

B, T, D = 4, 2048, 1024
NH, HDIM = 16, 64
GH = 8           # heads per core
DG = 512         # head dims per core
P = 128
NPAIR = 4        # head pairs per core
QB = 512         # q block width
NQB = T // QB
NKC = T // P     # k chunks of 128
NDC = D // P     # d chunks of 128
XW = 512         # x stream tile q-width
SCALE = 1.0 / np.sqrt(HDIM)

BF16 = mybir.dt.bfloat16
F32R = mybir.dt.float32r
F32 = mybir.dt.float32
AF = mybir.ActivationFunctionType

import os

_CACHE = {}
GP_EVAC = False   # GpSimd cannot read PSUM on TRN2; evacuations go to DVE
GP_BCAST = os.environ.get("K_GP_BCAST", "1") == "1"
# reciprocal_approx_fast returns NaNs on HW (works in CoreSim); keep off
FAST_RECIP = os.environ.get("K_FAST_RECIP", "0") == "1"
GP_WDMA = os.environ.get("K_GP_WDMA", "1") == "1"
# timing diagnostic: exp over 2 columns only + no mask (cripples correctness)
DIAG_TINY_EXP = os.environ.get("K_DIAG_TINY_EXP", "0") == "1"
GP_MUL = os.environ.get("K_GP_MUL", "0") == "1"


def _build(loop_n=None):
    nc = bacc.Bacc("TRN2", target_bir_lowering=False, debug=False, num_devices=8)
    xT = nc.dram_tensor("xt", [D, T], BF16, kind="ExternalInput").ap()
    wq = nc.dram_tensor("wq", [D, DG], BF16, kind="ExternalInput").ap()
    wk = nc.dram_tensor("wk", [D, DG], BF16, kind="ExternalInput").ap()
    wv = nc.dram_tensor("wv", [D, DG], BF16, kind="ExternalInput").ap()
    wo = nc.dram_tensor("wo", [DG, D], BF16, kind="ExternalInput").ap()
    tri = nc.dram_tensor("tri", [P, P], BF16, kind="ExternalInput").ap()
    ones = nc.dram_tensor("ones", [P, P], BF16, kind="ExternalInput").ap()
    out = nc.dram_tensor("out", [T, D], F32, kind="ExternalOutput").ap()

    xT_r = xT.rearrange("(dc p) q -> p dc q", p=P)
    wq_r = wq.rearrange("(dc p) n -> p dc n", p=P)
    wk_r = wk.rearrange("(dc p) n -> p dc n", p=P)
    wv_r = wv.rearrange("(dc p) n -> p dc n", p=P)
    wo_r = wo.rearrange("(c p) n -> p c n", p=P)
    out_r = out.rearrange("(qc p) n -> qc p n", p=P)

    with tile.TileContext(nc) as tc:
        with ExitStack() as top:
            pers = top.enter_context(tc.tile_pool(name="persist", bufs=1))
            qt_sb = pers.tile([P, NPAIR, T], BF16)
            kt_sb = pers.tile([P, NPAIR, T], BF16)
            v_sb = pers.tile([P, NKC, GH, HDIM + 1], BF16)
            ctxT = pers.tile([P, NPAIR, T], BF16)
            tri_sb = pers.tile([P, P], BF16)
            ones_sb = pers.tile([P, P], BF16)
            # all weights resident for the whole kernel (bf16 halves SBUF)
            wq_sb = pers.tile([P, NDC, DG], BF16)
            wk_sb = pers.tile([P, NDC, DG], BF16)
            wv_sb = pers.tile([P, NDC, DG], BF16)
            wo_sb = pers.tile([P, NPAIR, D], BF16)
            # weights ride separate DGE queues so they overlap each other and
            # the first x tile (sync queue) instead of serializing ahead of
            # it; wq/wk split per pair so the first projection group's slice
            # lands early
            wveng = nc.gpsimd if GP_WDMA else nc.scalar
            for pr in range(NPAIR):
                ps = slice(pr * P, (pr + 1) * P)
                nc.scalar.dma_start(wq_sb[:, :, ps], wq_r[:, :, ps])
            for pr in range(NPAIR):
                ps = slice(pr * P, (pr + 1) * P)
                nc.scalar.dma_start(wk_sb[:, :, ps], wk_r[:, :, ps])
            wveng.dma_start(wv_sb[:], wv_r)
            nc.scalar.dma_start(tri_sb[:], tri)
            nc.scalar.dma_start(ones_sb[:], ones)
            wveng.dma_start(wo_sb[:], wo_r)
            # ones-column of V (denominator trick)
            nc.vector.tensor_copy(
                v_sb[:, :, :, HDIM],
                ones_sb.rearrange("p (a b) -> p a b", a=NKC, b=GH),
            )

            body = ExitStack()
            xqp = body.enter_context(tc.tile_pool(name="xqp", bufs=2))
            # PSUM: pp (proj/V/out) 2 banks, st 2x2 banks, ctx 2 = 8 exactly.
            pp_psp = body.enter_context(
                tc.tile_pool(name="pp_ps", bufs=2, space="PSUM"))
            st_psp = body.enter_context(
                tc.tile_pool(name="st_ps", bufs=2, space="PSUM"))
            ctx_psp = body.enter_context(
                tc.tile_pool(name="ctx_ps", bufs=1, space="PSUM"))
            ptp = body.enter_context(tc.tile_pool(name="ptp", bufs=5))
            rcp = body.enter_context(tc.tile_pool(name="rcp", bufs=2))
            bcsp = body.enter_context(tc.tile_pool(name="bcsp", bufs=2))
            cup = body.enter_context(tc.tile_pool(name="cup", bufs=4))
            ostp = body.enter_context(tc.tile_pool(name="ost", bufs=3))

            eveng = nc.gpsimd if GP_EVAC else nc.vector
            xq_tiles = {}

            def emit_xdma(xi):
                xq = xqp.tile([P, NDC, XW], BF16, name="xq")
                nc.sync.dma_start(xq[:], xT_r[:, :, xi * XW:(xi + 1) * XW])
                xq_tiles[xi] = xq

            def acc_matmul(out, lhsT, rhs, start, stop):
                # concurrent row-tiled halves accumulating into the SAME
                # PSUM elements fault on HW (disjoint outputs are required
                # for tile_position concurrency) -- keep full-K matmuls
                nc.tensor.matmul(out, lhsT, rhs, start=start, stop=stop)

            def emit_proj_group(xi, w_sb, dst, pair):
                """One Q-or-K projection PSUM group (8 matmuls + evac)."""
                xq = xq_tiles[xi]
                pps = pp_psp.tile([P, XW], F32, name="pps")
                for dc in range(NDC):
                    acc_matmul(
                        pps[:],
                        w_sb[:, dc, pair * P:(pair + 1) * P],
                        xq[:, dc, :],
                        start=(dc == 0), stop=(dc == NDC - 1),
                    )
                with nc.allow_low_precision(reason="bf16 store"):
                    eveng.tensor_copy(
                        dst[:, pair, xi * XW:(xi + 1) * XW], pps[:])

            def emit_v_group(xi, kl):
                """One V-projection PSUM group (x-stationary)."""
                xq = xq_tiles[xi]
                kc = xi * (XW // P) + kl
                vps = pp_psp.tile([P, DG], F32, name="pps")
                for dc in range(NDC):
                    acc_matmul(
                        vps[:],
                        xq[:, dc, kl * P:(kl + 1) * P],
                        wv_sb[:, dc, :],
                        start=(dc == 0), stop=(dc == NDC - 1),
                    )
                with nc.allow_low_precision(reason="bf16 store"):
                    eveng.tensor_copy(
                        v_sb[:, kc, :, 0:HDIM],
                        vps.rearrange("p (h d) -> p h d", d=HDIM),
                    )

            def emit_out_group(qc, ob, ot):
                """Half of one out-projection row block (4 matmuls + evac)."""
                ops = pp_psp.tile([P, 512], F32, name="pps")
                for c in range(NPAIR):
                    acc_matmul(
                        ops[:],
                        ctxT[:, c, qc * P:(qc + 1) * P],
                        wo_sb[:, c, ob * 512:(ob + 1) * 512],
                        start=(c == 0), stop=(c == NPAIR - 1),
                    )
                eveng.tensor_copy(ot[:, ob * 512:(ob + 1) * 512], ops[:])
                if ob == 1:
                    nc.sync.dma_start(out_r[qc], ot[:])

            def proj_fillers(xi):
                """All PE filler groups that project q-block xi."""
                yield lambda: emit_xdma(xi)
                for w_sb, dst in ((wq_sb, qt_sb), (wk_sb, kt_sb)):
                    for pair in range(NPAIR):
                        yield lambda w=w_sb, d=dst, p=pair: emit_proj_group(
                            xi, w, d, p)
                for kl in range(XW // P):
                    yield lambda k=kl: emit_v_group(xi, k)

            def out_fillers(qb):
                """Out-projection fillers for q-block qb."""
                for ql in range(QB // P):
                    qc = qb * (QB // P) + ql
                    ot = ostp.tile([P, D], F32, name="ot")
                    for ob in range(2):
                        yield lambda q=qc, o=ob, t=ot: emit_out_group(q, o, t)

            def emit_chunk(qb, pair, kc, nkc, ctxp):
                r = P * kc - QB * qb
                lo = max(r, 0)
                st = st_psp.tile([P, 2, QB], F32, name="stps")
                pt = ptp.tile([P, 2, QB], BF16, name="pt")
                for hi in range(2):
                    # explicit tile_position: ~13x faster than the
                    # auto-derived row groups (HW-measured)
                    nc.tensor.matmul(
                        st[:, hi, lo:QB],
                        kt_sb[HDIM * hi:HDIM * (hi + 1), pair,
                              kc * P:(kc + 1) * P],
                        qt_sb[HDIM * hi:HDIM * (hi + 1), pair,
                              qb * QB + lo:(qb + 1) * QB],
                        start=True, stop=True,
                        tile_position=(HDIM * hi, 0),
                    )
                with nc.allow_low_precision(reason="bf16 probs"):
                    if DIAG_TINY_EXP:
                        nc.scalar.activation(
                            pt[:, :, lo:lo + 2], st[:, :, lo:lo + 2],
                            AF.Exp, scale=float(SCALE))
                    elif lo == 0:
                        # flat 2D APs run ~400ns faster on ACT than 3D
                        nc.scalar.activation(
                            pt.rearrange("p a b -> p (a b)"),
                            st.rearrange("p a b -> p (a b)"),
                            AF.Exp, scale=float(SCALE))
                    else:
                        nc.scalar.activation(
                            pt[:, :, lo:QB], st[:, :, lo:QB], AF.Exp,
                            scale=float(SCALE))
                if r >= 0 and not DIAG_TINY_EXP:
                    # one masked multiply for both heads (tri broadcast on
                    # the head dim via stride-0 AP)
                    with nc.allow_low_precision(reason="bf16 probs"):
                        nc.vector.tensor_tensor(
                            pt[:, :, r:r + P],
                            pt[:, :, r:r + P],
                            tri_sb[:].unsqueeze(1).to_broadcast([P, 2, P]),
                            mybir.AluOpType.mult,
                        )
                for hi in range(2):
                    acc_matmul(
                        ctxp[:, hi, lo:QB],
                        v_sb[:, kc, 2 * pair + hi, :],
                        pt[:, hi, lo:QB],
                        start=(kc == 0), stop=(kc == nkc - 1),
                    )

            def emit_pair_epilogue(qb, pair, ctxp):
                # one fp32 copy of [ctx|denom] for both heads frees the PSUM
                # banks; everything downstream reads SBUF, off the critical
                # path.  All ops merged across the two heads (flat APs).
                rdt = F32 if (GP_BCAST and FAST_RECIP) else F32R
                ctxu = cup.tile([HDIM + 1, 2, QB], F32, name="ctxu")
                nc.vector.tensor_copy(
                    ctxu.rearrange("p a b -> p (a b)"),
                    ctxp.rearrange("p a b -> p (a b)"))
                recip = rcp.tile([1, 2, QB], rdt, name="recip")
                den = ctxu[HDIM:HDIM + 1, :, :].rearrange("p a b -> p (a b)")
                if GP_BCAST and FAST_RECIP:
                    nc.vector.reciprocal_approx_fast(
                        recip.rearrange("p a b -> p (a b)"), den)
                else:
                    with nc.allow_low_precision(reason="recip"):
                        nc.vector.reciprocal(
                            recip.rearrange("p a b -> p (a b)"), den)
                bcs = bcsp.tile([HDIM, 2, QB], rdt, name="bcs")
                if GP_BCAST:
                    nc.gpsimd.partition_broadcast(
                        bcs.rearrange("p a b -> p (a b)"),
                        recip.rearrange("p a b -> p (a b)"))
                else:
                    for hi in range(2):
                        bc_ps = pp_psp.tile([P, QB], F32, name="pps")
                        nc.tensor.matmul(
                            bc_ps[:], ones_sb[0:1, :], recip[0:1, hi, :],
                            start=True, stop=True)
                        with nc.allow_low_precision(reason="f32r copy"):
                            nc.vector.tensor_copy(
                                bcs[:, hi, :], bc_ps[0:HDIM, :])
                muleng = nc.gpsimd if GP_MUL else nc.vector
                for hi in range(2):
                    with nc.allow_low_precision(reason="bf16 ctx"):
                        muleng.tensor_tensor(
                            ctxT[HDIM * hi:HDIM * (hi + 1), pair,
                                 qb * QB:(qb + 1) * QB],
                            ctxu[0:HDIM, hi, :],
                            bcs[:, hi, :],
                            mybir.AluOpType.mult,
                        )

            def emit_attention(qb, fillers):
                """Attention for q-block qb with PE fillers striped between
                chunks (the PE queue is in-order; fillers keep it busy while
                chunks wait on the ACT exp)."""
                nkc = (QB // P) * (qb + 1)
                nchunks = NPAIR * nkc
                acc, done = 0.0, 0
                ratio = len(fillers) / nchunks
                for pair in range(NPAIR):
                    ctxp = ctx_psp.tile([HDIM + 1, 2, QB], F32, name="ctxps")
                    for kc in range(nkc):
                        emit_chunk(qb, pair, kc, nkc, ctxp)
                        acc += ratio
                        while acc >= 1.0 and done < len(fillers):
                            fillers[done]()
                            done += 1
                            acc -= 1.0
                    emit_pair_epilogue(qb, pair, ctxp)
                while done < len(fillers):
                    fillers[done]()
                    done += 1

            lp = tc.For_i(0, loop_n, 1) if loop_n else nullcontext()
            with lp:
                # prologue: only pair 0's Q/K projections + V for q-block 0;
                # the remaining pairs ride as attention-qb0 fillers so the
                # exp stream starts ~14us earlier (pair p's chunks follow
                # pair p's projection fillers in emission order, and filler
                # pacing runs ~2 pairs ahead of consumption)
                emit_xdma(0)
                emit_proj_group(0, wq_sb, qt_sb, 0)
                emit_proj_group(0, wk_sb, kt_sb, 0)
                for kl in range(XW // P):
                    emit_v_group(0, kl)
                pro = []
                for pr in range(1, NPAIR):
                    pro.append(lambda p=pr: emit_proj_group(0, wq_sb, qt_sb, p))
                    pro.append(lambda p=pr: emit_proj_group(0, wk_sb, kt_sb, p))
                # out-proj fillers ride later iterations than strictly needed:
                # the attention tail (qb=3) is ACT-bound and needs the extra
                # PE filler, while early iterations are already PE-bound.
                out_sched = {2: [0], 3: [1, 2]}
                for xi in range(NQB):
                    fillers = pro if xi == 0 else []
                    if xi + 1 < NQB:
                        fillers.extend(proj_fillers(xi + 1))
                    for qb in out_sched.get(xi, []):
                        fillers.extend(out_fillers(qb))
                    emit_attention(xi, fillers)
                # epilogue: last q-block's output projection
                for f in out_fillers(NQB - 1):
                    f()
            body.close()

    nc.compile()
    return nc


def _get_nc():
    if "nc" not in _CACHE:
        _CACHE["nc"] = _build()
    return _CACHE["nc"]


def make_in_maps(inputs, W_q, W_k, W_v, W_o):
    import ml_dtypes
    bf16 = ml_dtypes.bfloat16
    x = np.asarray(inputs, dtype=np.float32)
    W_q = np.asarray(W_q, dtype=np.float32)
    W_k = np.asarray(W_k, dtype=np.float32)
    W_v = np.asarray(W_v, dtype=np.float32)
    W_o = np.asarray(W_o, dtype=np.float32)
    tri = np.where(
        np.arange(P)[:, None] <= np.arange(P)[None, :], 1.0, 0.0
    ).astype(bf16)
    ones = np.ones((P, P), dtype=bf16)
    in_maps = []
    for c in range(8):
        b, g = divmod(c, 2)
        gs = slice(g * DG, (g + 1) * DG)
        in_maps.append({
            "xt": np.ascontiguousarray(x[b].T).astype(bf16),
            "wq": np.ascontiguousarray(W_q[gs, :].T).astype(bf16),
            "wk": np.ascontiguousarray(W_k[gs, :].T).astype(bf16),
            "wv": np.ascontiguousarray(W_v[gs, :].T).astype(bf16),
            "wo": np.ascontiguousarray(W_o[:, gs].T).astype(bf16),
            "tri": tri,
            "ones": ones,
        })
    return in_maps


def combine(results, b_o):
    b_o = np.asarray(b_o, dtype=np.float32)
    out = np.empty((B, T, D), dtype=np.float32)
    for b in range(B):
        out[b] = results[2 * b]["out"] + results[2 * b + 1]["out"] + b_o
    return out


def kernel(inputs, W_q, W_k, W_v, W_o, b_o):
    nc = _get_nc()
    in_maps = make_in_maps(inputs, W_q, W_k, W_v, W_o)
    res = run_bass_kernel_spmd(nc, in_maps, core_ids=list(range(8)), trace=False)
    return combine(res.results, b_o)
